# revision 1
# baseline (speedup 1.0000x reference)
"""CVRP decoder kernel for Trainium2 (8 NeuronCores, batch-data-parallel).

Computes, per batch b (B=64, P=64, N=1000, H=128):
    q_graph   = mean_n(emb) @ Wq_graph
    q_first   = encoded_q1 @ Wq_first
    q_last    = emb[last_node] @ Wq_last
    q_visited = (vis01 @ emb / N) @ W_visited          (vis01 = isneginf(mask))
    final_q   = sum of the above + load*W_load + b_load
    score     = final_q @ emb^T / sqrt(H) - dists[last_node] / sqrt(2)
    probs     = softmax(10*tanh(score) + (-BIG if visited))

Sharding: batch dim across the 8 cores (pure data parallel), 8 batches per
core processed as 4 pairs of 2 batches stacked on the 128 SBUF partitions.
"""

import json
import math
import numpy as np
from contextlib import ExitStack

import concourse.bass as bass
import concourse.mybir as mybir
import concourse.tile as tile
from concourse.bass_utils import run_bass_kernel_spmd
from concourse.masks import make_identity


def _split_excess_waits(bir_bytes: bytes, max_waits: int = 1) -> bytes:
    """Walrus in this image rejects instructions carrying too many sem waits
    ("Too many sync wait commands", e.g. on Tile's kernel-tail Drain).
    Hoist excess waits onto preceding same-engine EventSemaphore carriers
    (pure sync ops) — sems are monotonic, so a chain of instructions whose
    waits partition the original list is equivalent."""
    d = json.loads(bir_bytes)
    n = [0]
    for fn in d.get("functions", []):
        for blk in fn.get("blocks", []):
            out = []
            for ins in blk.get("instructions", []):
                si = ins.get("sync_info") or {}
                waits = si.get("on_wait") or []
                if len(waits) > max_waits:
                    extra, keep = waits[:-max_waits], waits[-max_waits:]
                    ins["sync_info"]["on_wait"] = keep
                    for i in range(0, len(extra), max_waits):
                        n[0] += 1
                        carrier = {
                            "name": f"I-waitsplit-{n[0]}",
                            "opcode": "EventSemaphore",
                            "engine": ins["engine"],
                            "ins": [],
                            "outs": [],
                            "sync_info": {
                                "on_update": [],
                                "on_wait": extra[i:i + max_waits],
                            },
                        }
                        if "debug" in ins:
                            carrier["debug"] = ins["debug"]
                        out.append(carrier)
                out.append(ins)
            blk["instructions"] = out
    return json.dumps(d).encode()


def _install_walrus_shim():
    import concourse.bass2jax as b2j
    import concourse.bass_utils as bu
    if getattr(bu, "_waitsplit_installed", False):
        return
    real = bu.compile_bir_kernel

    def patched(bir_json, tmpdir, neff_name="file.neff", **kw):
        if isinstance(bir_json, (bytes, bytearray, str)):
            if isinstance(bir_json, str):
                bir_json = bir_json.encode()
            bir_json = _split_excess_waits(bir_json)
        return real(bir_json, tmpdir, neff_name=neff_name, **kw)

    bu.compile_bir_kernel = patched
    b2j.compile_bir_kernel = patched
    bu._waitsplit_installed = True


_install_walrus_shim()

F32 = mybir.dt.float32
I32 = mybir.dt.int32
OP = mybir.AluOpType
AF = mybir.ActivationFunctionType

B, P, N, H = 64, 64, 1000, 128
NCORES = 8
NB = B // NCORES          # 8 batches per core
NPAIR = NB // 2           # 4 pairs
NCHUNK = 8                # n-chunks of <=128 rows: 7*128 + 104
CHUNK_CNT = [128] * 7 + [N - 7 * 128]   # [128]*7 + [104]

MASK_NEG = -1000.0        # additive bias for visited nodes (pre x10 exp scale)
QV_SCALE = -1.0 / (1000.0 * N)   # undo MASK_NEG and the /N in one eviction
FQ_SCALE = math.sqrt(2.0) / math.sqrt(H)   # = 0.125 exactly
TANH_SCALE = 1.0 / math.sqrt(2.0)
TANH_CLIP = 10.0


def build_nc():
    nc = bass.Bass()

    dists = nc.dram_tensor("dists", [NB * N, N], F32, kind="ExternalInput")
    emb = nc.dram_tensor("emb", [NB * N, H], F32, kind="ExternalInput")
    eq1 = nc.dram_tensor("eq1", [NB * P, H], F32, kind="ExternalInput")
    lastnode = nc.dram_tensor("lastnode", [NB * P, 1], I32, kind="ExternalInput")
    loadv = nc.dram_tensor("loadv", [NPAIR, 128], F32, kind="ExternalInput")
    maskt = nc.dram_tensor("maskt", [NB * P, N], F32, kind="ExternalInput")
    wq_graph = nc.dram_tensor("wq_graph", [H, H], F32, kind="ExternalInput")
    wq_first = nc.dram_tensor("wq_first", [H, H], F32, kind="ExternalInput")
    wq_last = nc.dram_tensor("wq_last", [H, H], F32, kind="ExternalInput")
    w_visited = nc.dram_tensor("w_visited", [H, H], F32, kind="ExternalInput")
    w_load = nc.dram_tensor("w_load", [1, H], F32, kind="ExternalInput")
    b_load = nc.dram_tensor("b_load", [1, H], F32, kind="ExternalInput")
    probs = nc.dram_tensor("probs", [NB * P, N], F32, kind="ExternalOutput")

    with tile.TileContext(nc) as tc:
        with ExitStack() as ctx:
            const = ctx.enter_context(tc.tile_pool(name="const", bufs=1))
            sb = ctx.enter_context(tc.tile_pool(name="sb", bufs=3))
            sbe = ctx.enter_context(tc.tile_pool(name="sbe", bufs=4))
            ps_big = ctx.enter_context(
                tc.tile_pool(name="ps_big", bufs=4, space="PSUM"))
            ps_mb = ctx.enter_context(
                tc.tile_pool(name="ps_mb", bufs=2, space="PSUM"))
            ps_small = ctx.enter_context(
                tc.tile_pool(name="ps_small", bufs=2, space="PSUM"))

            # ---- constants ----
            ident = const.tile([128, 128], F32, tag="ident")
            make_identity(nc, ident[:])
            ones_row = const.tile([1, 128], F32, tag="ones_row")
            nc.gpsimd.memset(ones_row[:], 1.0)

            wg = const.tile([H, H], F32, tag="wg")
            nc.sync.dma_start(wg[:], wq_graph[:])
            wf = const.tile([H, H], F32, tag="wf")
            nc.sync.dma_start(wf[:], wq_first[:])
            wl = const.tile([H, H], F32, tag="wl")
            nc.sync.dma_start(wl[:], wq_last[:])
            wv = const.tile([H, H], F32, tag="wv")
            nc.sync.dma_start(wv[:], w_visited[:])
            wld = const.tile([1, H], F32, tag="wld")
            nc.sync.dma_start(wld[:], w_load[:])
            bld = const.tile([1, H], F32, tag="bld")
            nc.sync.dma_start(bld[:], b_load[:])

            for pr in range(NPAIR):
                b0 = 2 * pr              # first batch of the pair (core-local)
                r0 = 128 * pr            # row offset into [NB*P, ...] tensors

                # ---- indices: flat row index into [NB*N, ...] = idx + 1000*b
                idxr = sb.tile([128, 1], I32, tag="idxr")
                nc.sync.dma_start(idxr[:], lastnode[r0:r0 + 128, :])
                adj = sb.tile([128, 1], I32, tag="adj")
                nc.gpsimd.memset(adj[0:64, :], N * b0)
                nc.gpsimd.memset(adj[64:128, :], N * (b0 + 1))
                idxa = sb.tile([128, 1], I32, tag="idxa")
                nc.vector.tensor_tensor(out=idxa[:], in0=idxr[:], in1=adj[:],
                                        op=OP.add)

                # ---- gathers: dist rows + last-node embedding rows
                distg = sb.tile([128, N], F32, tag="distg")
                nc.gpsimd.indirect_dma_start(
                    out=distg[:], out_offset=None, in_=dists[:],
                    in_offset=bass.IndirectOffsetOnAxis(ap=idxa[:, 0:1], axis=0))
                lastemb = sb.tile([128, H], F32, tag="lastemb")
                nc.gpsimd.indirect_dma_start(
                    out=lastemb[:], out_offset=None, in_=emb[:],
                    in_offset=bass.IndirectOffsetOnAxis(ap=idxa[:, 0:1], axis=0))

                # ---- plain loads
                mk = sb.tile([128, N], F32, tag="mk")
                nc.sync.dma_start(mk[:], maskt[r0:r0 + 128, :])
                eq1s = sb.tile([128, H], F32, tag="eq1s")
                nc.sync.dma_start(eq1s[:], eq1[r0:r0 + 128, :])
                ldrow = sb.tile([1, 128], F32, tag="ldrow")
                nc.sync.dma_start(ldrow[:], loadv[pr:pr + 1, :])

                emb_n = []
                for j in range(2):
                    e = sbe.tile([128, NCHUNK, H], F32, tag="embn")
                    base = (b0 + j) * N
                    nc.sync.dma_start(e[:, 0:7, :],
                                      emb[base:base + 896, :]
                                      .rearrange("(c p) h -> p c h", p=128))
                    nc.sync.dma_start(e[0:104, 7, :], emb[base + 896:base + N, :])
                    emb_n.append(e)

                # ---- maskbias = (mask < -1e30) * (-1000)   {0, -1000}
                mb = sb.tile([128, N], F32, tag="mb")
                nc.vector.tensor_scalar(out=mb[:], in0=mk[:],
                                        scalar1=-1e30, scalar2=MASK_NEG,
                                        op0=OP.is_lt, op1=OP.mult)

                # ---- transpose maskbias -> mbT [n, 2p] chunks (PE, packed psum)
                mbT = sb.tile([128, NCHUNK, 128], F32, tag="mbT")
                for g in range(2):
                    pmb = ps_mb.tile([128, 4, 128], F32, tag="pmb")
                    for j in range(4):
                        c = 4 * g + j
                        cnt = CHUNK_CNT[c]
                        nc.tensor.transpose(
                            out=pmb[0:cnt, j, :],
                            in_=mb[:, 128 * c:128 * c + cnt],
                            identity=ident[:])
                    if g == 0:
                        nc.scalar.copy(mbT[:, 0:4, :], pmb[:])
                    else:
                        nc.scalar.copy(mbT[:, 4:7, :], pmb[:, 0:3, :])
                        nc.scalar.copy(mbT[0:104, 7, :], pmb[0:104, 3, :])

                # ---- embT per batch: [h, n] via PE transposes; mean via accum
                embT = []
                macc = []
                for j in range(2):
                    et = sbe.tile([128, 1024], F32, tag="embT")
                    acc = sb.tile([128, 2], F32, tag="macc")
                    psA = ps_big.tile([128, 512], F32, tag="psbig")
                    for c in range(4):
                        nc.tensor.transpose(
                            out=psA[:, 128 * c:128 * (c + 1)],
                            in_=emb_n[j][:, c, :],
                            identity=ident[:])
                    nc.scalar.activation(et[:, 0:512], psA[:], AF.Copy,
                                         accum_out=acc[:, 0:1])
                    psB = ps_big.tile([128, 488], F32, tag="psbig")
                    for c in range(4, NCHUNK):
                        cnt = CHUNK_CNT[c]
                        nc.tensor.transpose(
                            out=psB[:, 128 * c - 512:128 * c - 512 + cnt],
                            in_=emb_n[j][0:cnt, c, :],
                            identity=ident[0:cnt, 0:cnt])
                    nc.scalar.activation(et[:, 512:1000], psB[:], AF.Copy,
                                         accum_out=acc[:, 1:2])
                    embT.append(et)
                    macc.append(acc)

                # mean broadcast over the p dim: [128, 128]
                meanrep = sb.tile([128, 128], F32, tag="meanrep")
                for j in range(2):
                    ms = sb.tile([128, 1], F32, tag="ms")
                    nc.vector.tensor_tensor(out=ms[:], in0=macc[j][:, 0:1],
                                            in1=macc[j][:, 1:2], op=OP.add)
                    nc.vector.tensor_scalar(
                        out=meanrep[:, 64 * j:64 * j + 64],
                        in0=ms[:, 0:1].to_broadcast([128, 64]),
                        scalar1=1.0 / N, scalar2=None, op0=OP.mult)

                # ---- input transposes (whole pair at once)
                ps_t = ps_small.tile([128, 128], F32, tag="pss")
                nc.tensor.transpose(out=ps_t[:], in_=eq1s[:], identity=ident[:])
                eq1T = sb.tile([128, 128], F32, tag="eq1T")
                nc.vector.tensor_copy(out=eq1T[:], in_=ps_t[:])

                ps_t2 = ps_small.tile([128, 128], F32, tag="pss")
                nc.tensor.transpose(out=ps_t2[:], in_=lastemb[:],
                                    identity=ident[:])
                lastembT = sb.tile([128, 128], F32, tag="lastembT")
                nc.vector.tensor_copy(out=lastembT[:], in_=ps_t2[:])

                # ---- q_visited pre: psum[h, p] per batch
                qvs = sb.tile([128, 2, 64], F32, tag="qvs")
                for j in range(2):
                    pqv = ps_small.tile([128, 64], F32, tag="pss")
                    for c in range(NCHUNK):
                        cnt = CHUNK_CNT[c]
                        nc.tensor.matmul(
                            pqv[:],
                            lhsT=emb_n[j][0:cnt, c, :],
                            rhs=mbT[0:cnt, c, 64 * j:64 * j + 64],
                            start=(c == 0), stop=(c == NCHUNK - 1))
                    # psum = -1000 * sum_vis emb ; rescale to qv_pre/N
                    nc.vector.tensor_scalar(out=qvs[:, j, :], in0=pqv[:],
                                            scalar1=QV_SCALE, scalar2=None,
                                            op0=OP.mult)

                # ---- final_q^T accumulation: psum [h, 2p]
                pfq = ps_small.tile([128, 128], F32, tag="pss")
                nc.tensor.matmul(pfq[:], lhsT=wf[:], rhs=eq1T[:],
                                 start=True, stop=False)
                nc.tensor.matmul(pfq[:], lhsT=wl[:], rhs=lastembT[:],
                                 start=False, stop=False)
                nc.tensor.matmul(pfq[:], lhsT=wg[:], rhs=meanrep[:],
                                 start=False, stop=False)
                nc.tensor.matmul(pfq[:], lhsT=wv[:], rhs=qvs[:],
                                 start=False, stop=False)
                nc.tensor.matmul(pfq[:], lhsT=wld[:], rhs=ldrow[:],
                                 start=False, stop=False)
                nc.tensor.matmul(pfq[:], lhsT=bld[:], rhs=ones_row[:],
                                 start=False, stop=True)
                fqT = sb.tile([128, 128], F32, tag="fqT")
                nc.scalar.mul(fqT[:], pfq[:], FQ_SCALE)

                # ---- score matmuls + bias + tanh + mask + softmax
                u = sb.tile([128, N], F32, tag="u")
                for (n0, n1) in ((0, 512), (512, N)):
                    psc = ps_big.tile([128, n1 - n0], F32, tag="psbig")
                    for j in range(2):
                        nc.tensor.matmul(
                            psc[64 * j:64 * j + 64, :],
                            lhsT=fqT[:, 64 * j:64 * j + 64],
                            rhs=embT[j][:, n0:n1],
                            start=True, stop=True)
                    nc.vector.scalar_tensor_tensor(
                        out=u[:, n0:n1], in0=psc[:], scalar=0.0,
                        in1=distg[:, n0:n1],
                        op0=OP.bypass, op1=OP.subtract)

                t = sb.tile([128, N], F32, tag="t")
                nc.scalar.activation(t[:], u[:], AF.Tanh, scale=TANH_SCALE)
                w = sb.tile([128, N], F32, tag="w")
                nc.vector.tensor_tensor(out=w[:], in0=t[:], in1=mb[:], op=OP.add)

                e = sb.tile([128, N], F32, tag="e")
                ssum = sb.tile([128, 1], F32, tag="ssum")
                nc.scalar.activation(e[:], w[:], AF.Exp, scale=TANH_CLIP,
                                     accum_out=ssum[:])
                rec = sb.tile([128, 1], F32, tag="rec")
                nc.vector.reciprocal(out=rec[:], in_=ssum[:])
                pout = sb.tile([128, N], F32, tag="pout")
                nc.scalar.activation(pout[:], e[:], AF.Copy,
                                     scale=rec[:, 0:1])
                nc.sync.dma_start(probs[r0:r0 + 128, :], pout[:])

    return nc


_CACHE = {}


def _get_nc():
    if "nc" not in _CACHE:
        _CACHE["nc"] = build_nc()
    return _CACHE["nc"]


def _shard_inputs(inputs):
    dists = np.ascontiguousarray(inputs["dists"], dtype=np.float32)
    embeddings = np.ascontiguousarray(inputs["embeddings"], dtype=np.float32)
    encoded_q1 = np.ascontiguousarray(inputs["encoded_q1"], dtype=np.float32)
    last_node = np.ascontiguousarray(inputs["last_node"]).astype(np.int32)
    load = np.ascontiguousarray(inputs["load"], dtype=np.float32)
    mask = np.ascontiguousarray(inputs["group_ninf_mask"], dtype=np.float32)
    # -inf -> large finite negative: identical kernel behavior (the visited
    # test is `< -1e30`), but keeps every downstream ALU input finite.
    mask = np.maximum(mask, np.float32(-3e38))
    in_maps = []
    for c in range(NCORES):
        s = slice(c * NB, (c + 1) * NB)
        in_maps.append(dict(
            dists=dists[s].reshape(NB * N, N),
            emb=embeddings[s].reshape(NB * N, H),
            eq1=encoded_q1[s].reshape(NB * P, H),
            lastnode=last_node[s].reshape(NB * P, 1),
            loadv=load[s].reshape(NPAIR, 128),
            maskt=mask[s].reshape(NB * P, N),
            wq_graph=np.ascontiguousarray(inputs["Wq_graph"], dtype=np.float32),
            wq_first=np.ascontiguousarray(inputs["Wq_first"], dtype=np.float32),
            wq_last=np.ascontiguousarray(inputs["Wq_last"], dtype=np.float32),
            w_visited=np.ascontiguousarray(inputs["W_visited"], dtype=np.float32),
            w_load=np.ascontiguousarray(inputs["W_load"], dtype=np.float32)
                .reshape(1, H),
            b_load=np.ascontiguousarray(inputs["b_load"], dtype=np.float32)
                .reshape(1, H),
        ))
    return in_maps


def _run(inputs, trace=False, **kw):
    nc = _get_nc()
    in_maps = _shard_inputs(inputs)
    res = run_bass_kernel_spmd(nc, in_maps, list(range(NCORES)),
                               trace=trace, **kw)
    out = np.concatenate(
        [r["probs"].reshape(NB, P, N) for r in res.results], axis=0)
    return out, res


def kernel(**inputs) -> np.ndarray:
    out, _ = _run(inputs)
    return out



# revision 7
# speedup vs baseline: 1.9887x; 1.9887x over previous
"""CVRP decoder kernel for Trainium2 (8 NeuronCores, batch-data-parallel).

Computes, per batch b (B=64, P=64, N=1000, H=128):
    q_graph   = mean_n(emb) @ Wq_graph
    q_first   = encoded_q1 @ Wq_first
    q_last    = emb[last_node] @ Wq_last
    q_visited = (vis01 @ emb / N) @ W_visited          (vis01 = isneginf(mask))
    final_q   = sum of the above + load*W_load + b_load
    score     = final_q @ emb^T / sqrt(H) - dists[last_node] / sqrt(2)
    probs     = softmax(10*tanh(score) + (-BIG if visited))

Sharding: batch dim across the 8 cores (pure data parallel), 8 batches per
core processed as 4 pairs of 2 batches stacked on the 128 SBUF partitions.

v3: bf16 matmul path; host-pretiled/pretransposed layouts for dense DMA
packets and single-issue bulk loads; mask folded into the gathered distance
rows off the critical chain (tanh saturation makes exp(10*tanh) ~ 0 for
visited nodes, error ~1e-6 of scale); the mean rides the visited-sum matmul
via a ones-column; the whole working set is SBUF-resident and stages are
emitted in a pair/stage wavefront so the in-order engine queues pipeline
across pairs.
"""

import json
import math
import numpy as np
import ml_dtypes
from contextlib import ExitStack

import concourse.bass as bass
import concourse.mybir as mybir
import concourse.tile as tile
from concourse.bass_utils import run_bass_kernel_spmd
from concourse.masks import make_identity

BF16 = ml_dtypes.bfloat16


def _split_excess_waits(bir_bytes: bytes, max_waits: int = 1) -> bytes:
    """Walrus in this image rejects instructions carrying too many sem waits
    ("Too many sync wait commands", e.g. on Tile's kernel-tail Drain).
    Hoist excess waits onto preceding same-engine EventSemaphore carriers
    (pure sync ops) — sems are monotonic, so a chain of instructions whose
    waits partition the original list is equivalent."""
    d = json.loads(bir_bytes)
    n = [0]
    for fn in d.get("functions", []):
        for blk in fn.get("blocks", []):
            out = []
            for ins in blk.get("instructions", []):
                si = ins.get("sync_info") or {}
                waits = si.get("on_wait") or []
                if len(waits) > max_waits:
                    extra, keep = waits[:-max_waits], waits[-max_waits:]
                    ins["sync_info"]["on_wait"] = keep
                    for i in range(0, len(extra), max_waits):
                        n[0] += 1
                        carrier = {
                            "name": f"I-waitsplit-{n[0]}",
                            "opcode": "EventSemaphore",
                            "engine": ins["engine"],
                            "ins": [],
                            "outs": [],
                            "sync_info": {
                                "on_update": [],
                                "on_wait": extra[i:i + max_waits],
                            },
                        }
                        if "debug" in ins:
                            carrier["debug"] = ins["debug"]
                        out.append(carrier)
                out.append(ins)
            blk["instructions"] = out
    return json.dumps(d).encode()


def _install_walrus_shim():
    import concourse.bass2jax as b2j
    import concourse.bass_utils as bu
    if getattr(bu, "_waitsplit_installed", False):
        return
    real = bu.compile_bir_kernel

    def patched(bir_json, tmpdir, neff_name="file.neff", **kw):
        if isinstance(bir_json, (bytes, bytearray, str)):
            if isinstance(bir_json, str):
                bir_json = bir_json.encode()
            bir_json = _split_excess_waits(bir_json)
        return real(bir_json, tmpdir, neff_name=neff_name, **kw)

    bu.compile_bir_kernel = patched
    b2j.compile_bir_kernel = patched
    bu._waitsplit_installed = True


_install_walrus_shim()

F32 = mybir.dt.float32
F16 = mybir.dt.float16
BF = mybir.dt.bfloat16
I32 = mybir.dt.int32
OP = mybir.AluOpType
AF = mybir.ActivationFunctionType

B, P, N, H = 64, 64, 1000, 128
NCORES = 8
NB = B // NCORES          # 8 batches per core
NPAIR = NB // 2           # 4 pairs
NC = 8                    # n-chunks of 128 rows (last padded 104->128)
NPAD = NC * 128           # 1024

MASK_QV = -128.0          # maskT encoding for the visited-sum matmul
QV_SCALE = -1.0 / (128.0 * N)
MASK_PRE = 30.0           # added to dist rows: tanh(score - 21.2) -> -1
MEAN_SCALE = 1.0 / N
FQ_SCALE = math.sqrt(2.0) / math.sqrt(H)   # = 0.125 exactly
TANH_SCALE = 1.0 / math.sqrt(2.0)
TANH_CLIP = 10.0


def build_nc():
    nc = bass.Bass()

    dists = nc.dram_tensor("dists", [NB * N, N], F16, kind="ExternalInput")
    embt = nc.dram_tensor("embt", [NB * 128, NC * H], BF, kind="ExternalInput")
    embn = nc.dram_tensor("embn", [NB * N, H], BF, kind="ExternalInput")
    maskT = nc.dram_tensor("maskT", [128, NB * NC * 66], BF,
                           kind="ExternalInput")
    maskn = nc.dram_tensor("maskn", [128, NPAIR * N], F16,
                           kind="ExternalInput")
    eq1T = nc.dram_tensor("eq1T", [128, NPAIR * 128], BF, kind="ExternalInput")
    idxt = nc.dram_tensor("idxt", [128, NPAIR], I32, kind="ExternalInput")
    loadv = nc.dram_tensor("loadv", [1, NPAIR * 128], BF, kind="ExternalInput")
    w_all = nc.dram_tensor("w_all", [128, 4 * H], BF, kind="ExternalInput")
    w_lb = nc.dram_tensor("w_lb", [1, 2 * H], BF, kind="ExternalInput")
    probs = nc.dram_tensor("probs", [NB * P, N], BF, kind="ExternalOutput")

    with tile.TileContext(nc) as tc:
        with ExitStack() as ctx:
            const = ctx.enter_context(tc.tile_pool(name="const", bufs=1))
            sb = ctx.enter_context(tc.tile_pool(name="sb", bufs=1))
            ps_T = ctx.enter_context(
                tc.tile_pool(name="ps_T", bufs=3, space="PSUM"))
            ps_big = ctx.enter_context(
                tc.tile_pool(name="ps_big", bufs=2, space="PSUM"))
            ps_qv = ctx.enter_context(
                tc.tile_pool(name="ps_qv", bufs=2, space="PSUM"))
            ps_fq = ctx.enter_context(
                tc.tile_pool(name="ps_fq", bufs=1, space="PSUM"))

            # ---- constants / bulk params ----
            ident = const.tile([128, 128], BF, tag="ident")
            make_identity(nc, ident[:])
            ones_row = const.tile([1, 128], BF, tag="ones_row")
            nc.gpsimd.memset(ones_row[:], 1.0)
            wall = const.tile([128, 4, H], BF, tag="wall")
            nc.scalar.dma_start(wall[:], w_all[:].rearrange(
                "k (i h) -> k i h", i=4))
            wlb = const.tile([1, 2, H], BF, tag="wlb")
            nc.scalar.dma_start(wlb[:], w_lb[:].rearrange(
                "k (i h) -> k i h", i=2))
            idxs = const.tile([128, NPAIR], I32, tag="idxs")
            nc.gpsimd.dma_start(idxs[:], idxt[:])
            eqall = const.tile([128, NPAIR, 128], BF, tag="eqall")
            nc.scalar.dma_start(eqall[:], eq1T[:].rearrange(
                "k (p h) -> k p h", p=NPAIR))
            ldall = const.tile([1, NPAIR, 128], BF, tag="ldall")
            nc.scalar.dma_start(ldall[:], loadv[:].rearrange(
                "k (p h) -> k p h", p=NPAIR))

            # ---- per-pair persistent tiles ----
            et = [sb.tile([128, 2, NC, H], BF, tag=f"et{pr}", name=f"et{pr}")
                  for pr in range(NPAIR)]
            mT = [sb.tile([128, 2, NC, 66], BF, tag=f"mT{pr}", name=f"mT{pr}")
                  for pr in range(NPAIR)]
            mkn = [sb.tile([128, N], F16, tag=f"mkn{pr}", name=f"mkn{pr}")
                   for pr in range(NPAIR)]
            distg = [sb.tile([128, N], F16, tag=f"distg{pr}", name=f"distg{pr}")
                     for pr in range(NPAIR)]
            dmb = [sb.tile([128, N], F16, tag=f"dmb{pr}", name=f"dmb{pr}")
                   for pr in range(NPAIR)]
            lastemb = [sb.tile([128, H], BF, tag=f"lastemb{pr}", name=f"lastemb{pr}")
                       for pr in range(NPAIR)]
            eT = [[sb.tile([128, NPAD], BF, tag=f"eT{pr}_{j}", name=f"eT{pr}_{j}")
                   for j in range(2)] for pr in range(NPAIR)]
            fqr = [sb.tile([128, 2, 128], BF, tag=f"fqr{pr}", name=f"fqr{pr}")
                   for pr in range(NPAIR)]
            lastT = [sb.tile([128, 128], BF, tag=f"lastT{pr}", name=f"lastT{pr}")
                     for pr in range(NPAIR)]
            fqT = [sb.tile([128, 128], BF, tag=f"fqT{pr}", name=f"fqT{pr}")
                   for pr in range(NPAIR)]
            u = [sb.tile([128, N], F32, tag=f"u{pr}", name=f"u{pr}") for pr in range(NPAIR)]
            t = [sb.tile([128, N], F32, tag=f"t{pr}", name=f"t{pr}") for pr in range(NPAIR)]
            e = [sb.tile([128, N], F32, tag=f"e{pr}", name=f"e{pr}") for pr in range(NPAIR)]
            ssum = [sb.tile([128, 1], F32, tag=f"ssum{pr}", name=f"ssum{pr}")
                    for pr in range(NPAIR)]
            rec = [sb.tile([128, 1], F32, tag=f"rec{pr}", name=f"rec{pr}")
                   for pr in range(NPAIR)]
            pout = [sb.tile([128, N], BF, tag=f"pout{pr}", name=f"pout{pr}")
                    for pr in range(NPAIR)]

            def stage_load(pr):
                b0 = 2 * pr
                for j in range(2):
                    nc.sync.dma_start(
                        et[pr][:, j],
                        embt[(b0 + j) * 128:(b0 + j + 1) * 128, :]
                        .rearrange("k (c h) -> k c h", c=NC))
                nc.gpsimd.dma_start(
                    mT[pr][:],
                    maskT[:, b0 * NC * 66:(b0 + 2) * NC * 66]
                    .rearrange("k (j c h) -> k j c h", j=2, c=NC))
                nc.sync.dma_start(
                    mkn[pr][:], maskn[:, pr * N:(pr + 1) * N])
                nc.gpsimd.indirect_dma_start(
                    out=distg[pr][:], out_offset=None, in_=dists[:],
                    in_offset=bass.IndirectOffsetOnAxis(
                        ap=idxs[:, pr:pr + 1], axis=0))
                nc.gpsimd.indirect_dma_start(
                    out=lastemb[pr][:], out_offset=None, in_=embn[:],
                    in_offset=bass.IndirectOffsetOnAxis(
                        ap=idxs[:, pr:pr + 1], axis=0))

            def stage_transpose(pr):
                # dmb off-chain: dist rows + 30*visited
                nc.vector.tensor_tensor(out=dmb[pr][:], in0=distg[pr][:],
                                        in1=mkn[pr][:], op=OP.add)
                for j in range(2):
                    psA = ps_T.tile([128, 512], BF, tag="bigT")
                    for c in range(4):
                        nc.tensor.transpose(
                            out=psA[:, 128 * c:128 * (c + 1)],
                            in_=et[pr][:, j, c, :], identity=ident[:])
                    psB = ps_T.tile([128, 512], BF, tag="bigT")
                    for c in range(4, NC):
                        nc.tensor.transpose(
                            out=psB[:, 128 * (c - 4):128 * (c - 3)],
                            in_=et[pr][:, j, c, :], identity=ident[:])
                    if j == 0:
                        nc.vector.tensor_copy(out=eT[pr][j][:, 0:512],
                                              in_=psA[:])
                        nc.vector.tensor_copy(out=eT[pr][j][:, 512:1024],
                                              in_=psB[:])
                    else:
                        nc.scalar.copy(eT[pr][j][:, 0:512], psA[:])
                        nc.scalar.copy(eT[pr][j][:, 512:1024], psB[:])

            def stage_qv(pr):
                qv = ps_qv.tile([128, 2, 65], F32, tag="qv")
                for j in range(2):
                    for c in range(NC):
                        nc.tensor.matmul(
                            qv[:, j, :],
                            lhsT=et[pr][:, j, c, :],
                            rhs=mT[pr][:, j, c, 0:65],
                            start=(c == 0), stop=(c == NC - 1))
                for j in range(2):
                    nc.scalar.mul(fqr[pr][:, 1, 64 * j:64 * j + 64],
                                  qv[:, j, 0:64], QV_SCALE)
                    nc.vector.tensor_scalar(
                        out=fqr[pr][:, 0, 64 * j:64 * j + 64],
                        in0=qv[:, j, 64:65].to_broadcast([128, 64]),
                        scalar1=MEAN_SCALE, scalar2=None, op0=OP.mult)

            def stage_fq(pr):
                psT = ps_T.tile([128, 512], BF, tag="bigT")
                nc.tensor.transpose(out=psT[:, 0:128], in_=lastemb[pr][:],
                                    identity=ident[:])
                nc.vector.tensor_copy(out=lastT[pr][:], in_=psT[:, 0:128])
                pfq = ps_fq.tile([128, 128], F32, tag="pfq")
                nc.tensor.matmul(pfq[:], lhsT=wall[:, 0, :],
                                 rhs=eqall[:, pr, :], start=True, stop=False)
                nc.tensor.matmul(pfq[:], lhsT=wall[:, 1, :], rhs=lastT[pr][:],
                                 start=False, stop=False)
                nc.tensor.matmul(pfq[:], lhsT=wall[:, 2, :],
                                 rhs=fqr[pr][:, 0, :], start=False, stop=False)
                nc.tensor.matmul(pfq[:], lhsT=wall[:, 3, :],
                                 rhs=fqr[pr][:, 1, :], start=False, stop=False)
                nc.tensor.matmul(pfq[:], lhsT=wlb[:, 0, :],
                                 rhs=ldall[:, pr, :], start=False, stop=False)
                nc.tensor.matmul(pfq[:], lhsT=wlb[:, 1, :], rhs=ones_row[:],
                                 start=False, stop=True)
                nc.scalar.mul(fqT[pr][:], pfq[:], FQ_SCALE)

            def stage_score(pr):
                for (n0, n1) in ((0, 512), (512, N)):
                    psc = ps_big.tile([128, 512], F32, tag="big")
                    for j in range(2):
                        nc.tensor.matmul(
                            psc[64 * j:64 * j + 64, 0:n1 - n0],
                            lhsT=fqT[pr][:, 64 * j:64 * j + 64],
                            rhs=eT[pr][j][:, n0:n1],
                            start=True, stop=True)
                    nc.vector.scalar_tensor_tensor(
                        out=u[pr][:, n0:n1], in0=psc[:, 0:n1 - n0],
                        scalar=0.0, in1=dmb[pr][:, n0:n1],
                        op0=OP.bypass, op1=OP.subtract)

            def stage_exp(pr):
                nc.scalar.activation(t[pr][:], u[pr][:], AF.Tanh,
                                     scale=TANH_SCALE)
                nc.scalar.activation(e[pr][:], t[pr][:], AF.Exp,
                                     scale=TANH_CLIP, accum_out=ssum[pr][:])
                nc.vector.reciprocal(out=rec[pr][:], in_=ssum[pr][:])

            def stage_out(pr):
                nc.vector.tensor_scalar(
                    out=pout[pr][:], in0=e[pr][:], scalar1=rec[pr][:, 0:1],
                    scalar2=None, op0=OP.mult)
                nc.sync.dma_start(probs[128 * pr:128 * pr + 128, :],
                                  pout[pr][:])

            stages = [stage_load, stage_transpose, stage_qv, stage_fq,
                      stage_score, stage_exp, stage_out]
            for d in range(NPAIR + len(stages) - 1):
                for pr in range(NPAIR):
                    s = d - pr
                    if 0 <= s < len(stages):
                        stages[s](pr)

    return nc


_CACHE = {}


def _get_nc():
    if "nc" not in _CACHE:
        _CACHE["nc"] = build_nc()
    return _CACHE["nc"]


def _prep_inputs(inputs):
    """Host-side staging: dtype casts + DMA-friendly layouts (per full batch,
    then sliced per core)."""
    emb = np.ascontiguousarray(inputs["embeddings"], dtype=np.float32)
    emb_bf = emb.astype(BF16)                              # [B, N, H]
    # tiled: [B, 128, NC, H], row 128c+k -> [k, c], zero-padded past N
    emb_pad = np.zeros((B, NPAD, H), dtype=BF16)
    emb_pad[:, :N, :] = emb_bf
    embt = np.ascontiguousarray(
        emb_pad.reshape(B, NC, 128, H).transpose(0, 2, 1, 3))  # [B,128,NC,H]

    mask = np.ascontiguousarray(inputs["group_ninf_mask"], dtype=np.float32)
    vis = (mask < -1e30)
    enc = np.where(vis, np.float32(MASK_QV), np.float32(0)).astype(BF16)
    # transposed+tiled with ones column: [B, 128, NC, 66]
    enc_pad = np.zeros((B, P, NPAD), dtype=BF16)
    enc_pad[:, :, :N] = enc
    mt = enc_pad.reshape(B, P, NC, 128).transpose(0, 3, 2, 1)  # [B,128,NC,P]
    ones_col = np.zeros((B, 128, NC, 1), dtype=BF16)
    rowidx = np.arange(128)[:, None] + np.arange(NC)[None, :] * 128  # [128,NC]
    ones_col[:, :, :, 0] = (rowidx < N).astype(BF16)[None, :, :]
    zero_col = np.zeros((B, 128, NC, 1), dtype=BF16)
    maskTh = np.ascontiguousarray(
        np.concatenate([mt, ones_col, zero_col], axis=3))   # [B,128,NC,66]

    # pre-tanh mask bias, rides on the dist rows: {0, +MASK_PRE} fp16
    mknpre = np.where(vis, np.float16(MASK_PRE),
                      np.float16(0))                        # [B, P, N] f16

    q1 = np.ascontiguousarray(inputs["encoded_q1"], dtype=np.float32)
    q1_bf = q1.astype(BF16)                                 # [B, P, H]
    # per pair of batches: [h, 2*64]
    eq1T = np.ascontiguousarray(
        q1_bf.reshape(B // 2, 2, P, H).transpose(0, 3, 1, 2)
        .reshape(B // 2, H, 2 * P))                         # [B/2,128,128]

    last = np.ascontiguousarray(inputs["last_node"]).astype(np.int64)
    dists = np.ascontiguousarray(inputs["dists"], dtype=np.float32)
    dists_h = dists.astype(np.float16)                      # [B, N, N]

    load = np.ascontiguousarray(inputs["load"], dtype=np.float32).astype(BF16)

    w_bf = {k: np.ascontiguousarray(inputs[k], dtype=np.float32).astype(BF16)
            for k in ("Wq_graph", "Wq_first", "Wq_last", "W_visited",
                      "W_load", "b_load")}
    # [h, (wf, wl, wg, wv)] stacking for the single W_all tile
    w_stack = np.ascontiguousarray(np.stack(
        [w_bf["Wq_first"], w_bf["Wq_last"], w_bf["Wq_graph"],
         w_bf["W_visited"]], axis=1))                       # [H, 4, H]
    w_lb = np.ascontiguousarray(np.stack(
        [w_bf["W_load"], w_bf["b_load"]], axis=0))[None]    # [1, 2, H]

    in_maps = []
    for c in range(NCORES):
        s = slice(c * NB, (c + 1) * NB)
        lastc = last[s]                                     # [NB, P]
        # flat gather index within the core slab: n + N*local_b
        idx = (lastc + (np.arange(NB) * N)[:, None]).astype(np.int32)
        idxt = np.ascontiguousarray(
            idx.reshape(NPAIR, 128).T)                      # [128, NPAIR]
        in_maps.append(dict(
            dists=dists_h[s].reshape(NB * N, N),
            embt=embt[s].reshape(NB * 128, NC * H),
            embn=emb_bf[s].reshape(NB * N, H),
            maskT=np.ascontiguousarray(
                maskTh[s].transpose(1, 0, 2, 3)).reshape(128, NB * NC * 66),
            maskn=np.ascontiguousarray(
                mknpre[s].reshape(NPAIR, 128, N).transpose(1, 0, 2))
                .reshape(128, NPAIR * N),
            eq1T=np.ascontiguousarray(
                eq1T[c * NPAIR:(c + 1) * NPAIR].transpose(1, 0, 2))
                .reshape(128, NPAIR * 128),
            idxt=idxt,
            loadv=load[s].reshape(1, NPAIR * 128),
            w_all=w_stack.reshape(128, 4 * H),
            w_lb=w_lb.reshape(1, 2 * H),
        ))
    return in_maps


def _run(inputs, trace=False, **kw):
    nc = _get_nc()
    in_maps = _prep_inputs(inputs)
    res = run_bass_kernel_spmd(nc, in_maps, list(range(NCORES)),
                               trace=trace, **kw)
    out = np.concatenate(
        [np.asarray(r["probs"]).astype(np.float32).reshape(NB, P, N)
         for r in res.results], axis=0)
    return out, res


def kernel(**inputs) -> np.ndarray:
    out, _ = _run(inputs)
    return out


# revision 9
# speedup vs baseline: 2.0026x; 1.0070x over previous
"""CVRP decoder kernel for Trainium2 (8 NeuronCores, batch-data-parallel).

Computes, per batch b (B=64, P=64, N=1000, H=128):
    q_graph   = mean_n(emb) @ Wq_graph
    q_first   = encoded_q1 @ Wq_first
    q_last    = emb[last_node] @ Wq_last
    q_visited = (vis01 @ emb / N) @ W_visited          (vis01 = isneginf(mask))
    final_q   = sum of the above + load*W_load + b_load
    score     = final_q @ emb^T / sqrt(H) - dists[last_node] / sqrt(2)
    probs     = softmax(10*tanh(score) + (-BIG if visited))

Sharding: batch dim across the 8 cores (pure data parallel), 8 batches per
core processed as 4 pairs of 2 batches stacked on the 128 SBUF partitions.

v3: bf16 matmul path; host-pretiled/pretransposed layouts for dense DMA
packets and single-issue bulk loads; mask folded into the gathered distance
rows off the critical chain (tanh saturation makes exp(10*tanh) ~ 0 for
visited nodes, error ~1e-6 of scale); the mean rides the visited-sum matmul
via a ones-column; the whole working set is SBUF-resident and stages are
emitted in a pair/stage wavefront so the in-order engine queues pipeline
across pairs.
"""

import json
import math
import numpy as np
import ml_dtypes
from contextlib import ExitStack

import concourse.bass as bass
import concourse.mybir as mybir
import concourse.tile as tile
from concourse.bass_utils import run_bass_kernel_spmd
from concourse.masks import make_identity

BF16 = ml_dtypes.bfloat16


def _split_excess_waits(bir_bytes: bytes, max_waits: int = 1) -> bytes:
    """Walrus in this image rejects instructions carrying too many sem waits
    ("Too many sync wait commands", e.g. on Tile's kernel-tail Drain).
    Hoist excess waits onto preceding same-engine EventSemaphore carriers
    (pure sync ops) — sems are monotonic, so a chain of instructions whose
    waits partition the original list is equivalent."""
    d = json.loads(bir_bytes)
    n = [0]
    for fn in d.get("functions", []):
        for blk in fn.get("blocks", []):
            out = []
            for ins in blk.get("instructions", []):
                si = ins.get("sync_info") or {}
                waits = si.get("on_wait") or []
                if len(waits) > max_waits:
                    extra, keep = waits[:-max_waits], waits[-max_waits:]
                    ins["sync_info"]["on_wait"] = keep
                    for i in range(0, len(extra), max_waits):
                        n[0] += 1
                        carrier = {
                            "name": f"I-waitsplit-{n[0]}",
                            "opcode": "EventSemaphore",
                            "engine": ins["engine"],
                            "ins": [],
                            "outs": [],
                            "sync_info": {
                                "on_update": [],
                                "on_wait": extra[i:i + max_waits],
                            },
                        }
                        if "debug" in ins:
                            carrier["debug"] = ins["debug"]
                        out.append(carrier)
                out.append(ins)
            blk["instructions"] = out
    return json.dumps(d).encode()


def _install_walrus_shim():
    import concourse.bass2jax as b2j
    import concourse.bass_utils as bu
    if getattr(bu, "_waitsplit_installed", False):
        return
    real = bu.compile_bir_kernel

    def patched(bir_json, tmpdir, neff_name="file.neff", **kw):
        if isinstance(bir_json, (bytes, bytearray, str)):
            if isinstance(bir_json, str):
                bir_json = bir_json.encode()
            bir_json = _split_excess_waits(bir_json)
        return real(bir_json, tmpdir, neff_name=neff_name, **kw)

    bu.compile_bir_kernel = patched
    b2j.compile_bir_kernel = patched
    bu._waitsplit_installed = True


_install_walrus_shim()

F32 = mybir.dt.float32
F16 = mybir.dt.float16
BF = mybir.dt.bfloat16
I32 = mybir.dt.int32
OP = mybir.AluOpType
AF = mybir.ActivationFunctionType

B, P, N, H = 64, 64, 1000, 128
NCORES = 8
NB = B // NCORES          # 8 batches per core
NPAIR = NB // 2           # 4 pairs
NC = 8                    # n-chunks of 128 rows (last padded 104->128)
NPAD = NC * 128           # 1024

MASK_QV = -128.0          # maskT encoding for the visited-sum matmul
QV_SCALE = -1.0 / (128.0 * N)
MASK_PRE = 30.0           # added to dist rows: tanh(score - 21.2) -> -1
MEAN_SCALE = 1.0 / N
FQ_SCALE = math.sqrt(2.0) / math.sqrt(H)   # = 0.125 exactly
TANH_SCALE = 1.0 / math.sqrt(2.0)
TANH_CLIP = 10.0


def build_nc():
    nc = bass.Bass()

    dists = nc.dram_tensor("dists", [NB * N, N], F16, kind="ExternalInput")
    embt = nc.dram_tensor("embt", [NB * 128, NC * H], BF, kind="ExternalInput")
    embn = nc.dram_tensor("embn", [NB * N, H], BF, kind="ExternalInput")
    maskT = nc.dram_tensor("maskT", [128, NB * NC * 66], BF,
                           kind="ExternalInput")
    maskn = nc.dram_tensor("maskn", [128, NPAIR * N], F16,
                           kind="ExternalInput")
    eq1T = nc.dram_tensor("eq1T", [128, NPAIR * 128], BF, kind="ExternalInput")
    idxt = nc.dram_tensor("idxt", [128, NPAIR], I32, kind="ExternalInput")
    loadv = nc.dram_tensor("loadv", [1, NPAIR * 128], BF, kind="ExternalInput")
    w_all = nc.dram_tensor("w_all", [128, 4 * H], BF, kind="ExternalInput")
    w_lb = nc.dram_tensor("w_lb", [1, 2 * H], BF, kind="ExternalInput")
    probs = nc.dram_tensor("probs", [NB * P, N], BF, kind="ExternalOutput")

    with tile.TileContext(nc) as tc:
        with ExitStack() as ctx:
            const = ctx.enter_context(tc.tile_pool(name="const", bufs=1))
            sb = ctx.enter_context(tc.tile_pool(name="sb", bufs=1))
            ps_T = ctx.enter_context(
                tc.tile_pool(name="ps_T", bufs=3, space="PSUM"))
            ps_big = ctx.enter_context(
                tc.tile_pool(name="ps_big", bufs=2, space="PSUM"))
            ps_qv = ctx.enter_context(
                tc.tile_pool(name="ps_qv", bufs=2, space="PSUM"))
            ps_fq = ctx.enter_context(
                tc.tile_pool(name="ps_fq", bufs=1, space="PSUM"))

            # ---- constants / bulk params ----
            ident = const.tile([128, 128], BF, tag="ident")
            make_identity(nc, ident[:])
            ones_row = const.tile([1, 128], BF, tag="ones_row")
            nc.gpsimd.memset(ones_row[:], 1.0)
            wall = const.tile([128, 4, H], BF, tag="wall")
            nc.scalar.dma_start(wall[:], w_all[:].rearrange(
                "k (i h) -> k i h", i=4))
            wlb = const.tile([1, 2, H], BF, tag="wlb")
            nc.scalar.dma_start(wlb[:], w_lb[:].rearrange(
                "k (i h) -> k i h", i=2))
            idxs = const.tile([128, NPAIR], I32, tag="idxs")
            nc.gpsimd.dma_start(idxs[:], idxt[:])
            eqall = const.tile([128, NPAIR, 128], BF, tag="eqall")
            nc.scalar.dma_start(eqall[:], eq1T[:].rearrange(
                "k (p h) -> k p h", p=NPAIR))
            ldall = const.tile([1, NPAIR, 128], BF, tag="ldall")
            nc.scalar.dma_start(ldall[:], loadv[:].rearrange(
                "k (p h) -> k p h", p=NPAIR))

            # ---- per-pair persistent tiles ----
            et = [sb.tile([128, 2, NC, H], BF, tag=f"et{pr}", name=f"et{pr}")
                  for pr in range(NPAIR)]
            mT = [sb.tile([128, 2, NC, 66], BF, tag=f"mT{pr}", name=f"mT{pr}")
                  for pr in range(NPAIR)]
            dmb = [sb.tile([128, N], F16, tag=f"dmb{pr}", name=f"dmb{pr}")
                   for pr in range(NPAIR)]
            lastemb = [sb.tile([128, H], BF, tag=f"lastemb{pr}", name=f"lastemb{pr}")
                       for pr in range(NPAIR)]
            eT = [[sb.tile([128, NPAD], BF, tag=f"eT{pr}_{j}", name=f"eT{pr}_{j}")
                   for j in range(2)] for pr in range(NPAIR)]
            fqr = [sb.tile([128, 2, 128], BF, tag=f"fqr{pr}", name=f"fqr{pr}")
                   for pr in range(NPAIR)]
            lastT = [sb.tile([128, 128], BF, tag=f"lastT{pr}", name=f"lastT{pr}")
                     for pr in range(NPAIR)]
            fqT = [sb.tile([128, 128], BF, tag=f"fqT{pr}", name=f"fqT{pr}")
                   for pr in range(NPAIR)]
            u = [sb.tile([128, N], F32, tag=f"u{pr}", name=f"u{pr}") for pr in range(NPAIR)]
            t = [sb.tile([128, N], F32, tag=f"t{pr}", name=f"t{pr}") for pr in range(NPAIR)]
            e = [sb.tile([128, N], F32, tag=f"e{pr}", name=f"e{pr}") for pr in range(NPAIR)]
            ssum = [sb.tile([128, 1], F32, tag=f"ssum{pr}", name=f"ssum{pr}")
                    for pr in range(NPAIR)]
            rec = [sb.tile([128, 1], F32, tag=f"rec{pr}", name=f"rec{pr}")
                   for pr in range(NPAIR)]
            pout = [sb.tile([128, N], BF, tag=f"pout{pr}", name=f"pout{pr}")
                    for pr in range(NPAIR)]

            def stage_load(pr):
                b0 = 2 * pr
                for j in range(2):
                    nc.sync.dma_start(
                        et[pr][:, j],
                        embt[(b0 + j) * 128:(b0 + j + 1) * 128, :]
                        .rearrange("k (c h) -> k c h", c=NC))
                nc.gpsimd.dma_start(
                    mT[pr][:],
                    maskT[:, b0 * NC * 66:(b0 + 2) * NC * 66]
                    .rearrange("k (j c h) -> k j c h", j=2, c=NC))
                # mask bias lands in dmb, then the dist-row gather adds onto it
                nc.sync.dma_start(
                    dmb[pr][:], maskn[:, pr * N:(pr + 1) * N])
                nc.gpsimd.indirect_dma_start(
                    out=dmb[pr][:], out_offset=None, in_=dists[:],
                    in_offset=bass.IndirectOffsetOnAxis(
                        ap=idxs[:, pr:pr + 1], axis=0),
                    compute_op=OP.add)
                nc.gpsimd.indirect_dma_start(
                    out=lastemb[pr][:], out_offset=None, in_=embn[:],
                    in_offset=bass.IndirectOffsetOnAxis(
                        ap=idxs[:, pr:pr + 1], axis=0))

            def stage_transpose(pr):
                for j in range(2):
                    psA = ps_T.tile([128, 512], BF, tag="bigT")
                    for c in range(4):
                        nc.tensor.transpose(
                            out=psA[:, 128 * c:128 * (c + 1)],
                            in_=et[pr][:, j, c, :], identity=ident[:])
                    psB = ps_T.tile([128, 512], BF, tag="bigT")
                    for c in range(4, NC):
                        nc.tensor.transpose(
                            out=psB[:, 128 * (c - 4):128 * (c - 3)],
                            in_=et[pr][:, j, c, :], identity=ident[:])
                    if j == 0:
                        nc.vector.tensor_copy(out=eT[pr][j][:, 0:512],
                                              in_=psA[:])
                        nc.vector.tensor_copy(out=eT[pr][j][:, 512:1024],
                                              in_=psB[:])
                    else:
                        nc.scalar.copy(eT[pr][j][:, 0:512], psA[:])
                        nc.scalar.copy(eT[pr][j][:, 512:1024], psB[:])

            def stage_qv(pr):
                qv = ps_qv.tile([128, 2, 65], F32, tag="qv")
                for j in range(2):
                    for c in range(NC):
                        nc.tensor.matmul(
                            qv[:, j, :],
                            lhsT=et[pr][:, j, c, :],
                            rhs=mT[pr][:, j, c, 0:65],
                            start=(c == 0), stop=(c == NC - 1))
                nc.scalar.mul(
                    fqr[pr][:, 1, :].rearrange("k (j p) -> k j p", j=2),
                    qv[:, :, 0:64], QV_SCALE)
                nc.vector.tensor_scalar(
                    out=fqr[pr][:, 0, :].rearrange("k (j p) -> k j p", j=2),
                    in0=qv[:, :, 64:65].to_broadcast([128, 2, 64]),
                    scalar1=MEAN_SCALE, scalar2=None, op0=OP.mult)

            def stage_fq(pr):
                psT = ps_T.tile([128, 512], BF, tag="bigT")
                nc.tensor.transpose(out=psT[:, 0:128], in_=lastemb[pr][:],
                                    identity=ident[:])
                nc.vector.tensor_copy(out=lastT[pr][:], in_=psT[:, 0:128])
                pfq = ps_fq.tile([128, 128], F32, tag="pfq")
                nc.tensor.matmul(pfq[:], lhsT=wall[:, 0, :],
                                 rhs=eqall[:, pr, :], start=True, stop=False)
                nc.tensor.matmul(pfq[:], lhsT=wall[:, 1, :], rhs=lastT[pr][:],
                                 start=False, stop=False)
                nc.tensor.matmul(pfq[:], lhsT=wall[:, 2, :],
                                 rhs=fqr[pr][:, 0, :], start=False, stop=False)
                nc.tensor.matmul(pfq[:], lhsT=wall[:, 3, :],
                                 rhs=fqr[pr][:, 1, :], start=False, stop=False)
                nc.tensor.matmul(pfq[:], lhsT=wlb[:, 0, :],
                                 rhs=ldall[:, pr, :], start=False, stop=False)
                nc.tensor.matmul(pfq[:], lhsT=wlb[:, 1, :], rhs=ones_row[:],
                                 start=False, stop=True)
                nc.scalar.mul(fqT[pr][:], pfq[:], FQ_SCALE)

            def stage_score(pr):
                for (n0, n1) in ((0, 512), (512, N)):
                    psc = ps_big.tile([128, 512], F32, tag="big")
                    for j in range(2):
                        nc.tensor.matmul(
                            psc[64 * j:64 * j + 64, 0:n1 - n0],
                            lhsT=fqT[pr][:, 64 * j:64 * j + 64],
                            rhs=eT[pr][j][:, n0:n1],
                            start=True, stop=True)
                    nc.vector.scalar_tensor_tensor(
                        out=u[pr][:, n0:n1], in0=psc[:, 0:n1 - n0],
                        scalar=0.0, in1=dmb[pr][:, n0:n1],
                        op0=OP.bypass, op1=OP.subtract)

            def stage_exp(pr):
                nc.scalar.activation(t[pr][:], u[pr][:], AF.Tanh,
                                     scale=TANH_SCALE)
                nc.scalar.activation(e[pr][:], t[pr][:], AF.Exp,
                                     scale=TANH_CLIP, accum_out=ssum[pr][:])
                nc.vector.reciprocal(out=rec[pr][:], in_=ssum[pr][:])

            def stage_out(pr):
                nc.vector.tensor_scalar(
                    out=pout[pr][:], in0=e[pr][:], scalar1=rec[pr][:, 0:1],
                    scalar2=None, op0=OP.mult)
                nc.sync.dma_start(probs[128 * pr:128 * pr + 128, :],
                                  pout[pr][:])

            stages = [stage_load, stage_transpose, stage_qv, stage_fq,
                      stage_score, stage_exp, stage_out]
            for d in range(2 * (NPAIR - 1) + len(stages)):
                for pr in range(NPAIR):
                    s = d - 2 * pr
                    if 0 <= s < len(stages):
                        stages[s](pr)

    return nc


_CACHE = {}


def _get_nc():
    if "nc" not in _CACHE:
        _CACHE["nc"] = build_nc()
    return _CACHE["nc"]


def _prep_inputs(inputs):
    """Host-side staging: dtype casts + DMA-friendly layouts (per full batch,
    then sliced per core)."""
    emb = np.ascontiguousarray(inputs["embeddings"], dtype=np.float32)
    emb_bf = emb.astype(BF16)                              # [B, N, H]
    # tiled: [B, 128, NC, H], row 128c+k -> [k, c], zero-padded past N
    emb_pad = np.zeros((B, NPAD, H), dtype=BF16)
    emb_pad[:, :N, :] = emb_bf
    embt = np.ascontiguousarray(
        emb_pad.reshape(B, NC, 128, H).transpose(0, 2, 1, 3))  # [B,128,NC,H]

    mask = np.ascontiguousarray(inputs["group_ninf_mask"], dtype=np.float32)
    vis = (mask < -1e30)
    enc = np.where(vis, np.float32(MASK_QV), np.float32(0)).astype(BF16)
    # transposed+tiled with ones column: [B, 128, NC, 66]
    enc_pad = np.zeros((B, P, NPAD), dtype=BF16)
    enc_pad[:, :, :N] = enc
    mt = enc_pad.reshape(B, P, NC, 128).transpose(0, 3, 2, 1)  # [B,128,NC,P]
    ones_col = np.zeros((B, 128, NC, 1), dtype=BF16)
    rowidx = np.arange(128)[:, None] + np.arange(NC)[None, :] * 128  # [128,NC]
    ones_col[:, :, :, 0] = (rowidx < N).astype(BF16)[None, :, :]
    zero_col = np.zeros((B, 128, NC, 1), dtype=BF16)
    maskTh = np.ascontiguousarray(
        np.concatenate([mt, ones_col, zero_col], axis=3))   # [B,128,NC,66]

    # pre-tanh mask bias, rides on the dist rows: {0, +MASK_PRE} fp16
    mknpre = np.where(vis, np.float16(MASK_PRE),
                      np.float16(0))                        # [B, P, N] f16

    q1 = np.ascontiguousarray(inputs["encoded_q1"], dtype=np.float32)
    q1_bf = q1.astype(BF16)                                 # [B, P, H]
    # per pair of batches: [h, 2*64]
    eq1T = np.ascontiguousarray(
        q1_bf.reshape(B // 2, 2, P, H).transpose(0, 3, 1, 2)
        .reshape(B // 2, H, 2 * P))                         # [B/2,128,128]

    last = np.ascontiguousarray(inputs["last_node"]).astype(np.int64)
    dists = np.ascontiguousarray(inputs["dists"], dtype=np.float32)
    dists_h = dists.astype(np.float16)                      # [B, N, N]

    load = np.ascontiguousarray(inputs["load"], dtype=np.float32).astype(BF16)

    w_bf = {k: np.ascontiguousarray(inputs[k], dtype=np.float32).astype(BF16)
            for k in ("Wq_graph", "Wq_first", "Wq_last", "W_visited",
                      "W_load", "b_load")}
    # [h, (wf, wl, wg, wv)] stacking for the single W_all tile
    w_stack = np.ascontiguousarray(np.stack(
        [w_bf["Wq_first"], w_bf["Wq_last"], w_bf["Wq_graph"],
         w_bf["W_visited"]], axis=1))                       # [H, 4, H]
    w_lb = np.ascontiguousarray(np.stack(
        [w_bf["W_load"], w_bf["b_load"]], axis=0))[None]    # [1, 2, H]

    in_maps = []
    for c in range(NCORES):
        s = slice(c * NB, (c + 1) * NB)
        lastc = last[s]                                     # [NB, P]
        # flat gather index within the core slab: n + N*local_b
        idx = (lastc + (np.arange(NB) * N)[:, None]).astype(np.int32)
        idxt = np.ascontiguousarray(
            idx.reshape(NPAIR, 128).T)                      # [128, NPAIR]
        in_maps.append(dict(
            dists=dists_h[s].reshape(NB * N, N),
            embt=embt[s].reshape(NB * 128, NC * H),
            embn=emb_bf[s].reshape(NB * N, H),
            maskT=np.ascontiguousarray(
                maskTh[s].transpose(1, 0, 2, 3)).reshape(128, NB * NC * 66),
            maskn=np.ascontiguousarray(
                mknpre[s].reshape(NPAIR, 128, N).transpose(1, 0, 2))
                .reshape(128, NPAIR * N),
            eq1T=np.ascontiguousarray(
                eq1T[c * NPAIR:(c + 1) * NPAIR].transpose(1, 0, 2))
                .reshape(128, NPAIR * 128),
            idxt=idxt,
            loadv=load[s].reshape(1, NPAIR * 128),
            w_all=w_stack.reshape(128, 4 * H),
            w_lb=w_lb.reshape(1, 2 * H),
        ))
    return in_maps


def _run(inputs, trace=False, **kw):
    nc = _get_nc()
    in_maps = _prep_inputs(inputs)
    res = run_bass_kernel_spmd(nc, in_maps, list(range(NCORES)),
                               trace=trace, **kw)
    out = np.concatenate(
        [np.asarray(r["probs"]).astype(np.float32).reshape(NB, P, N)
         for r in res.results], axis=0)
    return out, res


def kernel(**inputs) -> np.ndarray:
    out, _ = _run(inputs)
    return out
